# revision 43
# baseline (speedup 1.0000x reference)
"""Differential attention kernel for 8 Trainium2 NeuronCores.

Sharding: batch x head-group. Core c handles batch b = c//4, heads
[4g, 4g+4) with g = c%4. Each core computes Q/K/V projections for its
heads over the full sequence, causal differential attention, and its
partial O-projection; the host sums the 4 partials per batch.

Differential attention trick: score = (q1.k1 - lam*q2.k2) * scale is a
single K=128 matmul with stacked [q1*scale; -lam*scale*q2] and [k1; k2]
head vectors (scales folded into the projection weights on the host).

Pipeline: heads are processed in sequence; while head h runs its
exp-bound attention on the Scalar engine, the PE executes head h+1's
Q/K projection chunks as filler, so the ~92us of exp hides behind
matmuls instead of serializing after them. Head 3's filler is the
O-projection of already-finished query chunks.

Softmax: scores are computed transposed (keys on partitions, queries
free), exp'd without max subtraction (inputs are bounded), and the
denominator comes from a ones-column appended to V in the P@V matmul.
Normalization per (head, chunk): DVE approx-reciprocal on the PSUM
denominator row, GpSimd partition-broadcast, DVE multiply. Causality
is structural (upper blocks skipped, diagonal score columns trimmed,
in-block triangle zeroed post-exp on DVE), which the host validates
against the attention_mask input before dispatch.
"""
import math
from collections import deque
from contextlib import ExitStack

import numpy as np
import ml_dtypes

S = 2048
H = 2048
NH = 16
HD = 64
NHC = 4          # heads per core
BF = ml_dtypes.bfloat16

_CACHED_NC = None
_DEBUG_TAPS = False


def _build_nc():
    import concourse.mybir as mybir
    import concourse.tile as tile
    from concourse import bacc

    bf16 = mybir.dt.bfloat16
    f32 = mybir.dt.float32
    AF = mybir.ActivationFunctionType

    nc = bacc.Bacc(None, target_bir_lowering=False)
    # inputs pre-arranged on host into partition-major layouts so every
    # DMA below is a contiguous stream
    hT = nc.declare_dram_parameter("hT", [128, S // 512, H // 128, 512], bf16, isOutput=False)
    wq = nc.declare_dram_parameter("wq", [128, NHC, H // 128, 128], bf16, isOutput=False)
    wk = nc.declare_dram_parameter("wk", [128, NHC, H // 128, 128], bf16, isOutput=False)
    wv = nc.declare_dram_parameter("wv", [128, H // 128, NHC * 65], bf16, isOutput=False)
    wo = nc.declare_dram_parameter("wo", [128, 2, S], bf16, isOutput=False)
    out = nc.declare_dram_parameter("out", [S, H], bf16, isOutput=True)
    if _DEBUG_TAPS:
        dbg_qT = nc.declare_dram_parameter("dbg_qT", [128, NHC, S], bf16, isOutput=True)
        dbg_kT = nc.declare_dram_parameter("dbg_kT", [128, NHC, S], bf16, isOutput=True)
        dbg_v4 = nc.declare_dram_parameter("dbg_v4", [128, S // 128, NHC * 65], bf16, isOutput=True)
        dbg_avt = nc.declare_dram_parameter("dbg_avt", [128, 2, S], bf16, isOutput=True)
        dbg_rec = nc.declare_dram_parameter("dbg_rec", [NHC * 4, 512], mybir.dt.float32, isOutput=True)
        dbg_den = nc.declare_dram_parameter("dbg_den", [NHC * 4, 512], mybir.dt.float32, isOutput=True)

    KT = H // 128    # 16 contraction tiles for projections
    NQ = S // 512    # 4 query chunks
    NS = S // 128    # 16 seq tiles

    with tile.TileContext(nc) as tc:
        with ExitStack() as ctx:
            # ---- persistent SBUF ----
            sb = ctx.enter_context(tc.tile_pool(name="sb", bufs=1))
            ht_all = sb.tile([128, KT, S], bf16)          # hidden^T
            wq_sb = sb.tile([128, NHC, KT, 128], bf16)
            wk_sb = sb.tile([128, NHC, KT, 128], bf16)
            wv_sb = sb.tile([128, KT, NHC * 65], bf16)
            wo_sb = sb.tile([128, 2, S], bf16)            # head-pair stacked Wo rows
            qT = sb.tile([128, NHC, S], bf16)             # [q1*s; -lam*s*q2] per head
            kT = sb.tile([128, NHC, S], bf16)             # [k1; k2] per head
            v4 = sb.tile([128, NS, NHC * 65], bf16)       # V tiles + ones cols
            avt = sb.tile([128, 2, S], bf16)              # attn_out^T, head pairs stacked
            # warm the ACT exp table while DMAs stream in
            warm = sb.tile([1, 16], f32)
            nc.vector.memset(warm[:], 0.0)
            nc.scalar.activation(warm[:], warm[:], AF.Exp)
            # lower-triangle (key <= query) 0/1 mask for diagonal blocks
            tri = sb.tile([128, 128], bf16)
            nc.gpsimd.memset(tri[:], 1.0)
            nc.gpsimd.affine_select(
                out=tri[:], in_=tri[:],
                compare_op=mybir.AluOpType.is_ge, fill=0.0,
                base=0, channel_multiplier=-1, pattern=[[1, 128]])

            # ---- input DMAs: few fat descriptors (Sync descriptor issue is
            # ~0.6us each and was pacing the whole projection ramp), ordered
            # by first use: head-0 weights, hidden k-groups, then the rest ----
            # hidden streams by query-column chunk: all 16 k-tiles of 512
            # queries per descriptor, so head-0's chunk-c projection AND
            # attention can start as soon as column chunk c lands
            nc.sync.dma_start(out=wq_sb[:, 0, :, :], in_=wq[:, 0, :, :])
            nc.sync.dma_start(out=wk_sb[:, 0, :, :], in_=wk[:, 0, :, :])
            nc.sync.dma_start(out=ht_all[:, :, 0:512], in_=hT[:, 0, :, :])
            nc.sync.dma_start(out=wv_sb[:], in_=wv[:, :, :])
            for c in range(1, NQ):
                nc.sync.dma_start(out=ht_all[:, :, c * 512:(c + 1) * 512],
                                  in_=hT[:, c, :, :])
            nc.sync.dma_start(out=wq_sb[:, 1, :, :], in_=wq[:, 1, :, :])
            nc.sync.dma_start(out=wk_sb[:, 1, :, :], in_=wk[:, 1, :, :])

            pt_pool = ctx.enter_context(tc.tile_pool(name="ptw", bufs=4))
            nrm_work = ctx.enter_context(tc.tile_pool(name="nrmw", bufs=2))
            orow_pool = ctx.enter_context(tc.tile_pool(name="orow", bufs=4))

            atp = ctx.enter_context(tc.tile_pool(name="atp", bufs=2, space="PSUM"))
            wkp = ctx.enter_context(tc.tile_pool(name="wkp", bufs=2, space="PSUM"))

            # warm the PE clock (HAM) with junk matmuls while DMAs land;
            # first on `warm` (ready earliest), then on tri
            wj = wkp.tile([128, 512], f32, tag="work", name="warmp")
            for _ in range(24):
                nc.tensor.matmul(wj[0:16, 0:16], lhsT=warm[:], rhs=warm[:],
                                 start=True, stop=True)
            for _ in range(84):
                nc.tensor.matmul(wj[:, 0:128], lhsT=tri[:], rhs=tri[:],
                                 start=True, stop=True)

            def v_unit(st):
                def run():
                    vp = wkp.tile([128, 512], f32, tag="work")
                    for k in range(KT):
                        nc.tensor.matmul(vp[:, 0:NHC * 65],
                                         lhsT=ht_all[:, k, st * 128:(st + 1) * 128],
                                         rhs=wv_sb[:, k, :],
                                         start=(k == 0), stop=(k == KT - 1))
                    nc.vector.tensor_copy(v4[:, st, :], vp[:, 0:NHC * 65])
                    for j in range(NHC):
                        nc.gpsimd.memset(v4[:, st, j * 65 + 64:j * 65 + 65], 1.0)
                return run

            def proj_unit(h, w_sb, dst, nj, evac=0):
                # two half-contraction pieces so filler slots stay fine-grained
                box = {}

                def run_a():
                    pp = wkp.tile([128, 512], f32, tag="work",
                                  name=f"pp{h}_{nj}_{0 if dst is qT else 1}")
                    box["pp"] = pp
                    for k in range(KT // 2):
                        nc.tensor.matmul(pp[:], lhsT=w_sb[:, h, k, :],
                                         rhs=ht_all[:, k, nj * 512:(nj + 1) * 512],
                                         start=(k == 0), stop=False)

                def run_b():
                    pp = box["pp"]
                    for k in range(KT // 2, KT):
                        nc.tensor.matmul(pp[:], lhsT=w_sb[:, h, k, :],
                                         rhs=ht_all[:, k, nj * 512:(nj + 1) * 512],
                                         start=False, stop=(k == KT - 1))
                    if evac:
                        nc.scalar.copy(dst[:, h, nj * 512:(nj + 1) * 512], pp[:])
                    else:
                        nc.vector.tensor_copy(dst[:, h, nj * 512:(nj + 1) * 512], pp[:])
                return run_a, run_b

            def oproj_unit(qi, nch, evac):
                def run():
                    op = wkp.tile([128, 512], f32, tag="work")
                    for p in range(2):
                        nc.tensor.matmul(op[:],
                                         lhsT=avt[:, p, qi * 128:(qi + 1) * 128],
                                         rhs=wo_sb[:, p, nch * 512:(nch + 1) * 512],
                                         start=(p == 0), stop=(p == 1))
                    orow = orows[qi]
                    if evac == 0:
                        nc.vector.tensor_copy(orow[:, nch * 512:(nch + 1) * 512], op[:])
                    else:
                        nc.scalar.copy(orow[:, nch * 512:(nch + 1) * 512], op[:])
                    done = orow_filled[qi] = orow_filled[qi] + 1
                    if done == 2:
                        nc.sync.dma_start(out=out[qi * 128:(qi + 1) * 128, 0:1024],
                                          in_=orow[:, 0:1024])
                    elif done == NQ:
                        nc.sync.dma_start(out=out[qi * 128:(qi + 1) * 128, 1024:2048],
                                          in_=orow[:, 1024:2048])
                return run

            orows = {}
            orow_filled = {}
            filler = deque()

            def attn_chunk(h, nj, slot_cb):
                qs = slice(nj * 512, (nj + 1) * 512)
                nblk = 4 * nj + 4
                av = atp.tile([65, 512], f32, tag="av", name=f"av{h}_{nj}")
                pv_lag = deque()   # (spans, pt) awaiting P@V, depth 2
                for kg in range(nblk // 2):
                    sc = atp.tile([128, 1024], f32, tag="sc", name=f"sc{h}_{nj}_{kg}")
                    pt = pt_pool.tile([128, 1024], bf16, tag="pt")
                    spans = []
                    for u in range(2):
                        ki = 2 * kg + u
                        uu = ki - 4 * nj  # >=0 on diagonal blocks
                        g0 = u * 512 + max(uu, 0) * 128
                        g1 = (u + 1) * 512
                        nc.tensor.matmul(sc[:, g0:g1],
                                         lhsT=kT[:, h, ki * 128:(ki + 1) * 128],
                                         rhs=qT[:, h, nj * 512 + max(uu, 0) * 128:(nj + 1) * 512],
                                         start=True, stop=True)
                        spans.append((g0, g1, uu))
                    if spans[0][1] == spans[1][0] == 512 and spans[0][0] == 0:
                        nc.scalar.activation(pt[:, 0:1024], sc[:, 0:1024], AF.Exp)
                    else:
                        for g0, g1, _ in spans:
                            nc.scalar.activation(pt[:, g0:g1], sc[:, g0:g1], AF.Exp)
                    for g0, _, uu in spans:
                        if uu >= 0:
                            # zero the in-block upper triangle (key > query)
                            nc.gpsimd.affine_select(
                                out=pt[:, g0:g0 + 128],
                                in_=pt[:, g0:g0 + 128],
                                compare_op=mybir.AluOpType.is_ge,
                                fill=0.0,
                                base=0,
                                channel_multiplier=-1,
                                pattern=[[1, 128]],
                            )
                    if len(pv_lag) == 2:
                        lag_spans, lag_pt = pv_lag.popleft()
                        for g0, g1, uu, ki in lag_spans:
                            nc.tensor.matmul(av[:, max(uu, 0) * 128:512],
                                             lhsT=v4[:, ki, h * 65:(h + 1) * 65],
                                             rhs=lag_pt[:, g0:g1],
                                             start=(ki == 0), stop=(ki == nblk - 1))
                    slot_cb()
                    pv_lag.append(([(g0, g1, uu, 2 * kg + i)
                                    for i, (g0, g1, uu) in enumerate(spans)], pt))
                while pv_lag:
                    lag_spans, lag_pt = pv_lag.popleft()
                    for g0, g1, uu, ki in lag_spans:
                        nc.tensor.matmul(av[:, max(uu, 0) * 128:512],
                                         lhsT=v4[:, ki, h * 65:(h + 1) * 65],
                                         rhs=lag_pt[:, g0:g1],
                                         start=(ki == 0), stop=(ki == nblk - 1))
                # normalize: row 64 of av is the softmax denominator
                den = nrm_work.tile([1, 512], f32, tag="den")
                nc.scalar.copy(den[:], av[64:65, :])
                rec = nrm_work.tile([1, 512], f32, tag="rec")
                nc.vector.reciprocal_approx_fast(rec[:], den[:])
                if _DEBUG_TAPS:
                    nc.sync.dma_start(out=dbg_rec[h * 4 + nj:h * 4 + nj + 1, :], in_=rec[:])
                    dent = nrm_work.tile([1, 512], f32, tag="dent")
                    nc.vector.tensor_copy(dent[:], av[64:65, :])
                    nc.sync.dma_start(out=dbg_den[h * 4 + nj:h * 4 + nj + 1, :], in_=dent[:])
                bcs = nrm_work.tile([64, 512], f32, tag="bcs")
                nc.gpsimd.partition_broadcast(bcs[:], rec[:])
                pair, odd = divmod(h, 2)
                nc.vector.tensor_mul(avt[64 * odd:64 * odd + 64, pair, qs],
                                     av[0:64, :], bcs[:])
                slot_cb()

            # ---- head 0: column-streamed ramp. Chunk-c projection, V
            # blocks, and attention run as soon as ht column-chunk c lands;
            # next column's units fill this chunk's attention slots ----
            for fn in proj_unit(0, wq_sb, qT, 0, evac=1):
                fn()
            for fn in proj_unit(0, wk_sb, kT, 0):
                fn()
            for st in range(4):
                v_unit(st)()
            for c in range(NQ):
                if c + 1 < NQ:
                    filler.extend(proj_unit(0, wq_sb, qT, c + 1, evac=1))
                    filler.extend(proj_unit(0, wk_sb, kT, c + 1))
                    for st in range(4 * (c + 1), 4 * (c + 1) + 4):
                        filler.append(v_unit(st))
                else:
                    # head-2 weights stream during head-0 attention
                    nc.sync.dma_start(out=wq_sb[:, 2, :, :], in_=wq[:, 2, :, :])
                    nc.sync.dma_start(out=wk_sb[:, 2, :, :], in_=wk[:, 2, :, :])
                    for w_sb, dst in ((wq_sb, qT), (wk_sb, kT)):
                        for nj in range(NQ):
                            filler.extend(proj_unit(1, w_sb, dst, nj))
                cslots = [2 * c + 3]

                def c_cb(cslots=cslots):
                    n = 0 if not filler else -(-len(filler) // max(1, cslots[0]))
                    for _ in range(min(n, len(filler))):
                        filler.popleft()()
                    cslots[0] -= 1
                attn_chunk(0, c, c_cb)

            # ---- heads 1-3 ----
            TOT_SLOTS = sum(2 * nj + 2 + 1 for nj in range(NQ))
            for h in range(1, NHC):
                # deferred weight streams off the ramp-critical window
                if h == 1:
                    nc.sync.dma_start(out=wq_sb[:, 3, :, :], in_=wq[:, 3, :, :])
                    nc.sync.dma_start(out=wk_sb[:, 3, :, :], in_=wk[:, 3, :, :])
                    nc.sync.dma_start(out=wo_sb[:], in_=wo[:, :, :])
                # queue next head's Q/K projection as PE filler under this
                # head's exp-bound attention
                if h + 1 < NHC:
                    for w_sb, dst in ((wq_sb, qT), (wk_sb, kT)):
                        for nj in range(NQ):
                            filler.extend(proj_unit(h + 1, w_sb, dst, nj))
                n0 = len(filler)
                plan = [0] * TOT_SLOTS
                for i in range(n0):
                    plan[i * TOT_SLOTS // n0] += 1
                state = [0, TOT_SLOTS]

                def h_cb(h=h, plan=plan, state=state):
                    slot_no, slots = state
                    n = plan[slot_no]
                    if h == NHC - 1 and filler:
                        n += -(-len(filler) // max(1, slots))
                    for _ in range(min(n, len(filler))):
                        filler.popleft()()
                    state[0] += 1
                    state[1] -= 1
                for nj in range(NQ):
                    attn_chunk(h, nj, h_cb)
                    if h == NHC - 1:
                        # O-projection of this finished chunk becomes filler
                        for qi in range(4 * nj, 4 * nj + 4):
                            orows[qi] = orow_pool.tile([128, S], bf16, tag="orow",
                                                       name=f"or{qi}")
                            orow_filled[qi] = 0
                            for nch in range(NQ):
                                ev = 1 if nj == NQ - 1 and (qi + nch) % 2 == 0 else 0
                                filler.append(oproj_unit(qi, nch, ev))
            while filler:
                filler.popleft()()
            if _DEBUG_TAPS:
                nc.sync.dma_start(out=dbg_qT[:], in_=qT[:])
                nc.sync.dma_start(out=dbg_kT[:], in_=kT[:])
                nc.sync.dma_start(out=dbg_v4[:], in_=v4[:])
                nc.sync.dma_start(out=dbg_avt[:], in_=avt[:])
    return nc


def _get_nc():
    global _CACHED_NC
    if _CACHED_NC is None:
        nc = _build_nc()
        if not nc.is_finalized():
            nc.finalize()
        _CACHED_NC = nc
    return _CACHED_NC


def _prep_in_maps(hidden_states, Wq, Wk, Wv, Wo, lambda_param):
    lam = math.tanh(math.log1p(math.exp(float(lambda_param))))
    scale = HD ** -0.5
    in_maps = []
    # [p, col-chunk, k, s]: DMA-contiguous per 512-query column chunk
    hTb = [np.ascontiguousarray(
        hidden_states[b].T.reshape(16, 128, 4, 512).transpose(1, 2, 0, 3)
    ).astype(BF) for b in range(2)]
    for core in range(8):
        b, g = divmod(core, 4)
        heads = range(NHC * g, NHC * g + NHC)
        wq_cols, wk_cols = [], []
        for h in heads:
            wq_cols.append(Wq[:, h * 64:(h + 1) * 64] * scale)
            wq_cols.append(Wq[:, (NH + h) * 64:(NH + h + 1) * 64] * (-lam * scale))
            wk_cols.append(Wk[:, h * 64:(h + 1) * 64])
            wk_cols.append(Wk[:, (NH + h) * 64:(NH + h + 1) * 64])
        wv_pad = np.zeros((H, NHC * 65), dtype=np.float32)
        for j, h in enumerate(heads):
            wv_pad[:, j * 65:j * 65 + 64] = Wv[:, h * 64:(h + 1) * 64]
        heads = list(heads)
        wo_sel = np.zeros((128, 2, S), dtype=np.float32)  # head-pair stacked rows
        for p in range(2):
            h0, h1 = heads[2 * p], heads[2 * p + 1]
            wo_sel[0:64, p] = Wo[h0 * 64:(h0 + 1) * 64, :]
            wo_sel[64:128, p] = Wo[h1 * 64:(h1 + 1) * 64, :]
        wqa = np.concatenate(wq_cols, axis=1)
        wka = np.concatenate(wk_cols, axis=1)
        in_maps.append({
            "hT": hTb[b],
            "wq": np.ascontiguousarray(
                wqa.reshape(16, 128, 4, 128).transpose(1, 2, 0, 3)).astype(BF),
            "wk": np.ascontiguousarray(
                wka.reshape(16, 128, 4, 128).transpose(1, 2, 0, 3)).astype(BF),
            "wv": np.ascontiguousarray(
                wv_pad.reshape(16, 128, 260).transpose(1, 0, 2)).astype(BF),
            "wo": np.ascontiguousarray(wo_sel).astype(BF),
        })
    return in_maps


def _mask_is_causal(attention_mask):
    m = np.asarray(attention_mask)
    if m.shape != (2, 1, S, S):
        return False
    neg = np.float32(np.finfo(np.float32).min)
    tri = np.tril(np.ones((S, S), dtype=bool))
    expect = np.where(tri, np.float32(0.0), neg)
    return all(np.array_equal(m[b, 0], expect) for b in range(m.shape[0]))


def _fallback(hidden_states, attention_mask, Wq, Wk, Wv, Wo, lambda_param):
    hs = hidden_states.astype(np.float32)
    lam = math.tanh(math.log1p(math.exp(float(lambda_param))))
    scaling = HD ** -0.5
    B = hs.shape[0]
    out = np.empty((B, S, H), dtype=np.float32)
    for b in range(B):
        q_all = (hs[b] @ Wq).reshape(S, 2 * NH, HD).transpose(1, 0, 2)
        k_all = (hs[b] @ Wk).reshape(S, 2 * NH, HD).transpose(1, 0, 2)
        v = (hs[b] @ Wv).reshape(S, NH, HD).transpose(1, 0, 2)
        acc = np.zeros((S, H), dtype=np.float32)
        for h in range(NH):
            s1 = q_all[h] @ k_all[h].T
            s2 = q_all[NH + h] @ k_all[NH + h].T
            sc = (s1 - lam * s2) * scaling + attention_mask[b, 0]
            sc -= sc.max(axis=-1, keepdims=True)
            p = np.exp(sc)
            p /= p.sum(axis=-1, keepdims=True)
            acc += (p @ v[h]) @ Wo[h * 64:(h + 1) * 64]
        out[b] = acc
    return out


def _run(inputs, trace=False):
    from concourse.bass_utils import run_bass_kernel_spmd

    hidden_states = np.asarray(inputs["hidden_states"], dtype=np.float32)
    attention_mask = np.asarray(inputs["attention_mask"], dtype=np.float32)
    Wq = np.asarray(inputs["Wq"], dtype=np.float32)
    Wk = np.asarray(inputs["Wk"], dtype=np.float32)
    Wv = np.asarray(inputs["Wv"], dtype=np.float32)
    Wo = np.asarray(inputs["Wo"], dtype=np.float32)
    lam_p = inputs["lambda_param"]

    if not _mask_is_causal(attention_mask):
        return _fallback(hidden_states, attention_mask, Wq, Wk, Wv, Wo, lam_p), None

    in_maps = _prep_in_maps(hidden_states, Wq, Wk, Wv, Wo, lam_p)
    nc = _get_nc()
    res = run_bass_kernel_spmd(nc, in_maps, list(range(8)), trace=trace)
    out = np.empty((2, S, H), dtype=np.float32)
    for b in range(2):
        acc = res.results[4 * b]["out"].astype(np.float32)
        for g in range(1, 4):
            acc = acc + res.results[4 * b + g]["out"].astype(np.float32)
        out[b] = acc
    return out, res


def kernel(**inputs):
    out, _ = _run(inputs, trace=False)
    return out


# revision 44
# speedup vs baseline: 1.0008x; 1.0008x over previous
"""Differential attention kernel for 8 Trainium2 NeuronCores.

Sharding: batch x head-group. Core c handles batch b = c//4, heads
[4g, 4g+4) with g = c%4. Each core computes Q/K/V projections for its
heads over the full sequence, causal differential attention, and its
partial O-projection; the host sums the 4 partials per batch.

Differential attention trick: score = (q1.k1 - lam*q2.k2) * scale is a
single K=128 matmul with stacked [q1*scale; -lam*scale*q2] and [k1; k2]
head vectors (scales folded into the projection weights on the host).

Pipeline: heads are processed in sequence; while head h runs its
exp-bound attention on the Scalar engine, the PE executes head h+1's
Q/K projection chunks as filler, so the ~92us of exp hides behind
matmuls instead of serializing after them. Head 3's filler is the
O-projection of already-finished query chunks.

Softmax: scores are computed transposed (keys on partitions, queries
free), exp'd without max subtraction (inputs are bounded), and the
denominator comes from a ones-column appended to V in the P@V matmul.
Normalization per (head, chunk): DVE approx-reciprocal on the PSUM
denominator row, GpSimd partition-broadcast, DVE multiply. Causality
is structural (upper blocks skipped, diagonal score columns trimmed,
in-block triangle zeroed post-exp on DVE), which the host validates
against the attention_mask input before dispatch.
"""
import math
from collections import deque
from contextlib import ExitStack

import numpy as np
import ml_dtypes

S = 2048
H = 2048
NH = 16
HD = 64
NHC = 4          # heads per core
BF = ml_dtypes.bfloat16

_CACHED_NC = None
_DEBUG_TAPS = False


def _build_nc():
    import concourse.mybir as mybir
    import concourse.tile as tile
    from concourse import bacc

    bf16 = mybir.dt.bfloat16
    f32 = mybir.dt.float32
    AF = mybir.ActivationFunctionType

    nc = bacc.Bacc(None, target_bir_lowering=False)
    # inputs pre-arranged on host into partition-major layouts so every
    # DMA below is a contiguous stream
    hT = nc.declare_dram_parameter("hT", [128, S // 512, H // 128, 512], bf16, isOutput=False)
    wq = nc.declare_dram_parameter("wq", [128, NHC, H // 128, 128], bf16, isOutput=False)
    wk = nc.declare_dram_parameter("wk", [128, NHC, H // 128, 128], bf16, isOutput=False)
    wv = nc.declare_dram_parameter("wv", [128, H // 128, NHC * 65], bf16, isOutput=False)
    wo = nc.declare_dram_parameter("wo", [128, 2, S], bf16, isOutput=False)
    out = nc.declare_dram_parameter("out", [S, H], bf16, isOutput=True)
    if _DEBUG_TAPS:
        dbg_qT = nc.declare_dram_parameter("dbg_qT", [128, NHC, S], bf16, isOutput=True)
        dbg_kT = nc.declare_dram_parameter("dbg_kT", [128, NHC, S], bf16, isOutput=True)
        dbg_v4 = nc.declare_dram_parameter("dbg_v4", [128, S // 128, NHC * 65], bf16, isOutput=True)
        dbg_avt = nc.declare_dram_parameter("dbg_avt", [128, 2, S], bf16, isOutput=True)
        dbg_rec = nc.declare_dram_parameter("dbg_rec", [NHC * 4, 512], mybir.dt.float32, isOutput=True)
        dbg_den = nc.declare_dram_parameter("dbg_den", [NHC * 4, 512], mybir.dt.float32, isOutput=True)

    KT = H // 128    # 16 contraction tiles for projections
    NQ = S // 512    # 4 query chunks
    NS = S // 128    # 16 seq tiles

    with tile.TileContext(nc) as tc:
        with ExitStack() as ctx:
            # ---- persistent SBUF ----
            sb = ctx.enter_context(tc.tile_pool(name="sb", bufs=1))
            ht_all = sb.tile([128, KT, S], bf16)          # hidden^T
            wq_sb = sb.tile([128, NHC, KT, 128], bf16)
            wk_sb = sb.tile([128, NHC, KT, 128], bf16)
            wv_sb = sb.tile([128, KT, NHC * 65], bf16)
            wo_sb = sb.tile([128, 2, S], bf16)            # head-pair stacked Wo rows
            qT = sb.tile([128, NHC, S], bf16)             # [q1*s; -lam*s*q2] per head
            kT = sb.tile([128, NHC, S], bf16)             # [k1; k2] per head
            v4 = sb.tile([128, NS, NHC * 65], bf16)       # V tiles + ones cols
            avt = sb.tile([128, 2, S], bf16)              # attn_out^T, head pairs stacked
            # warm the ACT exp table while DMAs stream in
            warm = sb.tile([1, 16], f32)
            nc.vector.memset(warm[:], 0.0)
            nc.scalar.activation(warm[:], warm[:], AF.Exp)
            # lower-triangle (key <= query) 0/1 mask for diagonal blocks
            tri = sb.tile([128, 128], bf16)
            nc.gpsimd.memset(tri[:], 1.0)
            nc.gpsimd.affine_select(
                out=tri[:], in_=tri[:],
                compare_op=mybir.AluOpType.is_ge, fill=0.0,
                base=0, channel_multiplier=-1, pattern=[[1, 128]])

            # ---- input DMAs: few fat descriptors (Sync descriptor issue is
            # ~0.6us each and was pacing the whole projection ramp), ordered
            # by first use: head-0 weights, hidden k-groups, then the rest ----
            # hidden streams by query-column chunk: all 16 k-tiles of 512
            # queries per descriptor, so head-0's chunk-c projection AND
            # attention can start as soon as column chunk c lands
            nc.sync.dma_start(out=wq_sb[:, 0, :, :], in_=wq[:, 0, :, :])
            nc.sync.dma_start(out=wk_sb[:, 0, :, :], in_=wk[:, 0, :, :])
            nc.sync.dma_start(out=ht_all[:, :, 0:512], in_=hT[:, 0, :, :])
            nc.sync.dma_start(out=wv_sb[:], in_=wv[:, :, :])
            for c in range(1, NQ):
                nc.sync.dma_start(out=ht_all[:, :, c * 512:(c + 1) * 512],
                                  in_=hT[:, c, :, :])
            nc.sync.dma_start(out=wq_sb[:, 1, :, :], in_=wq[:, 1, :, :])
            nc.sync.dma_start(out=wk_sb[:, 1, :, :], in_=wk[:, 1, :, :])

            pt_pool = ctx.enter_context(tc.tile_pool(name="ptw", bufs=4))
            nrm_work = ctx.enter_context(tc.tile_pool(name="nrmw", bufs=2))
            orow_pool = ctx.enter_context(tc.tile_pool(name="orow", bufs=4))

            atp = ctx.enter_context(tc.tile_pool(name="atp", bufs=2, space="PSUM"))
            wkp = ctx.enter_context(tc.tile_pool(name="wkp", bufs=2, space="PSUM"))

            # warm the PE clock (HAM) with junk matmuls while DMAs land;
            # first on `warm` (ready earliest), then on tri
            wj = wkp.tile([128, 512], f32, tag="work", name="warmp")
            for _ in range(24):
                nc.tensor.matmul(wj[0:16, 0:16], lhsT=warm[:], rhs=warm[:],
                                 start=True, stop=True)
            for _ in range(44):
                nc.tensor.matmul(wj[:, 0:128], lhsT=tri[:], rhs=tri[:],
                                 start=True, stop=True)

            def v_unit(st):
                def run():
                    vp = wkp.tile([128, 512], f32, tag="work")
                    for k in range(KT):
                        nc.tensor.matmul(vp[:, 0:NHC * 65],
                                         lhsT=ht_all[:, k, st * 128:(st + 1) * 128],
                                         rhs=wv_sb[:, k, :],
                                         start=(k == 0), stop=(k == KT - 1))
                    nc.vector.tensor_copy(v4[:, st, :], vp[:, 0:NHC * 65])
                    for j in range(NHC):
                        nc.gpsimd.memset(v4[:, st, j * 65 + 64:j * 65 + 65], 1.0)
                return run

            def proj_unit(h, w_sb, dst, nj, evac=0):
                # two half-contraction pieces so filler slots stay fine-grained
                box = {}

                def run_a():
                    pp = wkp.tile([128, 512], f32, tag="work",
                                  name=f"pp{h}_{nj}_{0 if dst is qT else 1}")
                    box["pp"] = pp
                    for k in range(KT // 2):
                        nc.tensor.matmul(pp[:], lhsT=w_sb[:, h, k, :],
                                         rhs=ht_all[:, k, nj * 512:(nj + 1) * 512],
                                         start=(k == 0), stop=False)

                def run_b():
                    pp = box["pp"]
                    for k in range(KT // 2, KT):
                        nc.tensor.matmul(pp[:], lhsT=w_sb[:, h, k, :],
                                         rhs=ht_all[:, k, nj * 512:(nj + 1) * 512],
                                         start=False, stop=(k == KT - 1))
                    if evac:
                        nc.scalar.copy(dst[:, h, nj * 512:(nj + 1) * 512], pp[:])
                    else:
                        nc.vector.tensor_copy(dst[:, h, nj * 512:(nj + 1) * 512], pp[:])
                return run_a, run_b

            def oproj_unit(qi, nch, evac):
                def run():
                    op = wkp.tile([128, 512], f32, tag="work")
                    for p in range(2):
                        nc.tensor.matmul(op[:],
                                         lhsT=avt[:, p, qi * 128:(qi + 1) * 128],
                                         rhs=wo_sb[:, p, nch * 512:(nch + 1) * 512],
                                         start=(p == 0), stop=(p == 1))
                    orow = orows[qi]
                    if evac == 0:
                        nc.vector.tensor_copy(orow[:, nch * 512:(nch + 1) * 512], op[:])
                    else:
                        nc.scalar.copy(orow[:, nch * 512:(nch + 1) * 512], op[:])
                    done = orow_filled[qi] = orow_filled[qi] + 1
                    if done == 2:
                        nc.sync.dma_start(out=out[qi * 128:(qi + 1) * 128, 0:1024],
                                          in_=orow[:, 0:1024])
                    elif done == NQ:
                        nc.sync.dma_start(out=out[qi * 128:(qi + 1) * 128, 1024:2048],
                                          in_=orow[:, 1024:2048])
                return run

            orows = {}
            orow_filled = {}
            filler = deque()

            def attn_chunk(h, nj, slot_cb):
                qs = slice(nj * 512, (nj + 1) * 512)
                nblk = 4 * nj + 4
                av = atp.tile([65, 512], f32, tag="av", name=f"av{h}_{nj}")
                pv_lag = deque()   # (spans, pt) awaiting P@V, depth 2
                for kg in range(nblk // 2):
                    sc = atp.tile([128, 1024], f32, tag="sc", name=f"sc{h}_{nj}_{kg}")
                    pt = pt_pool.tile([128, 1024], bf16, tag="pt")
                    spans = []
                    for u in range(2):
                        ki = 2 * kg + u
                        uu = ki - 4 * nj  # >=0 on diagonal blocks
                        g0 = u * 512 + max(uu, 0) * 128
                        g1 = (u + 1) * 512
                        nc.tensor.matmul(sc[:, g0:g1],
                                         lhsT=kT[:, h, ki * 128:(ki + 1) * 128],
                                         rhs=qT[:, h, nj * 512 + max(uu, 0) * 128:(nj + 1) * 512],
                                         start=True, stop=True)
                        spans.append((g0, g1, uu))
                    if spans[0][1] == spans[1][0] == 512 and spans[0][0] == 0:
                        nc.scalar.activation(pt[:, 0:1024], sc[:, 0:1024], AF.Exp)
                    else:
                        for g0, g1, _ in spans:
                            nc.scalar.activation(pt[:, g0:g1], sc[:, g0:g1], AF.Exp)
                    for g0, _, uu in spans:
                        if uu >= 0:
                            # zero the in-block upper triangle (key > query)
                            nc.gpsimd.affine_select(
                                out=pt[:, g0:g0 + 128],
                                in_=pt[:, g0:g0 + 128],
                                compare_op=mybir.AluOpType.is_ge,
                                fill=0.0,
                                base=0,
                                channel_multiplier=-1,
                                pattern=[[1, 128]],
                            )
                    if len(pv_lag) == 2:
                        lag_spans, lag_pt = pv_lag.popleft()
                        for g0, g1, uu, ki in lag_spans:
                            nc.tensor.matmul(av[:, max(uu, 0) * 128:512],
                                             lhsT=v4[:, ki, h * 65:(h + 1) * 65],
                                             rhs=lag_pt[:, g0:g1],
                                             start=(ki == 0), stop=(ki == nblk - 1))
                    slot_cb()
                    pv_lag.append(([(g0, g1, uu, 2 * kg + i)
                                    for i, (g0, g1, uu) in enumerate(spans)], pt))
                while pv_lag:
                    lag_spans, lag_pt = pv_lag.popleft()
                    for g0, g1, uu, ki in lag_spans:
                        nc.tensor.matmul(av[:, max(uu, 0) * 128:512],
                                         lhsT=v4[:, ki, h * 65:(h + 1) * 65],
                                         rhs=lag_pt[:, g0:g1],
                                         start=(ki == 0), stop=(ki == nblk - 1))
                # normalize: row 64 of av is the softmax denominator
                den = nrm_work.tile([1, 512], f32, tag="den")
                nc.scalar.copy(den[:], av[64:65, :])
                rec = nrm_work.tile([1, 512], f32, tag="rec")
                nc.vector.reciprocal_approx_fast(rec[:], den[:])
                if _DEBUG_TAPS:
                    nc.sync.dma_start(out=dbg_rec[h * 4 + nj:h * 4 + nj + 1, :], in_=rec[:])
                    dent = nrm_work.tile([1, 512], f32, tag="dent")
                    nc.vector.tensor_copy(dent[:], av[64:65, :])
                    nc.sync.dma_start(out=dbg_den[h * 4 + nj:h * 4 + nj + 1, :], in_=dent[:])
                bcs = nrm_work.tile([64, 512], f32, tag="bcs")
                nc.gpsimd.partition_broadcast(bcs[:], rec[:])
                pair, odd = divmod(h, 2)
                nc.vector.tensor_mul(avt[64 * odd:64 * odd + 64, pair, qs],
                                     av[0:64, :], bcs[:])
                slot_cb()

            # ---- head 0: column-streamed ramp. Chunk-c projection, V
            # blocks, and attention run as soon as ht column-chunk c lands;
            # next column's units fill this chunk's attention slots ----
            for fn in proj_unit(0, wq_sb, qT, 0, evac=1):
                fn()
            for fn in proj_unit(0, wk_sb, kT, 0):
                fn()
            for st in range(4):
                v_unit(st)()
            for c in range(NQ):
                if c + 1 < NQ:
                    filler.extend(proj_unit(0, wq_sb, qT, c + 1, evac=1))
                    filler.extend(proj_unit(0, wk_sb, kT, c + 1))
                    for st in range(4 * (c + 1), 4 * (c + 1) + 4):
                        filler.append(v_unit(st))
                else:
                    # head-2 weights stream during head-0 attention
                    nc.sync.dma_start(out=wq_sb[:, 2, :, :], in_=wq[:, 2, :, :])
                    nc.sync.dma_start(out=wk_sb[:, 2, :, :], in_=wk[:, 2, :, :])
                    for w_sb, dst in ((wq_sb, qT), (wk_sb, kT)):
                        for nj in range(NQ):
                            filler.extend(proj_unit(1, w_sb, dst, nj))
                cslots = [2 * c + 3]

                def c_cb(cslots=cslots):
                    n = 0 if not filler else -(-len(filler) // max(1, cslots[0]))
                    for _ in range(min(n, len(filler))):
                        filler.popleft()()
                    cslots[0] -= 1
                attn_chunk(0, c, c_cb)

            # ---- heads 1-3 ----
            TOT_SLOTS = sum(2 * nj + 2 + 1 for nj in range(NQ))
            for h in range(1, NHC):
                # deferred weight streams off the ramp-critical window
                if h == 1:
                    nc.sync.dma_start(out=wq_sb[:, 3, :, :], in_=wq[:, 3, :, :])
                    nc.sync.dma_start(out=wk_sb[:, 3, :, :], in_=wk[:, 3, :, :])
                    nc.sync.dma_start(out=wo_sb[:], in_=wo[:, :, :])
                # queue next head's Q/K projection as PE filler under this
                # head's exp-bound attention
                if h + 1 < NHC:
                    for w_sb, dst in ((wq_sb, qT), (wk_sb, kT)):
                        for nj in range(NQ):
                            filler.extend(proj_unit(h + 1, w_sb, dst, nj))
                n0 = len(filler)
                plan = [0] * TOT_SLOTS
                for i in range(n0):
                    plan[i * TOT_SLOTS // n0] += 1
                state = [0, TOT_SLOTS]

                def h_cb(h=h, plan=plan, state=state):
                    slot_no, slots = state
                    n = plan[slot_no]
                    if h == NHC - 1 and filler:
                        n += -(-len(filler) // max(1, slots))
                    for _ in range(min(n, len(filler))):
                        filler.popleft()()
                    state[0] += 1
                    state[1] -= 1
                for nj in range(NQ):
                    attn_chunk(h, nj, h_cb)
                    if h == NHC - 1:
                        # O-projection of this finished chunk becomes filler
                        for qi in range(4 * nj, 4 * nj + 4):
                            orows[qi] = orow_pool.tile([128, S], bf16, tag="orow",
                                                       name=f"or{qi}")
                            orow_filled[qi] = 0
                            for nch in range(NQ):
                                ev = 1 if nj == NQ - 1 and (qi + nch) % 2 == 0 else 0
                                filler.append(oproj_unit(qi, nch, ev))
            while filler:
                filler.popleft()()
            if _DEBUG_TAPS:
                nc.sync.dma_start(out=dbg_qT[:], in_=qT[:])
                nc.sync.dma_start(out=dbg_kT[:], in_=kT[:])
                nc.sync.dma_start(out=dbg_v4[:], in_=v4[:])
                nc.sync.dma_start(out=dbg_avt[:], in_=avt[:])
    return nc


def _get_nc():
    global _CACHED_NC
    if _CACHED_NC is None:
        nc = _build_nc()
        if not nc.is_finalized():
            nc.finalize()
        _CACHED_NC = nc
    return _CACHED_NC


def _prep_in_maps(hidden_states, Wq, Wk, Wv, Wo, lambda_param):
    lam = math.tanh(math.log1p(math.exp(float(lambda_param))))
    scale = HD ** -0.5
    in_maps = []
    # [p, col-chunk, k, s]: DMA-contiguous per 512-query column chunk
    hTb = [np.ascontiguousarray(
        hidden_states[b].T.reshape(16, 128, 4, 512).transpose(1, 2, 0, 3)
    ).astype(BF) for b in range(2)]
    for core in range(8):
        b, g = divmod(core, 4)
        heads = range(NHC * g, NHC * g + NHC)
        wq_cols, wk_cols = [], []
        for h in heads:
            wq_cols.append(Wq[:, h * 64:(h + 1) * 64] * scale)
            wq_cols.append(Wq[:, (NH + h) * 64:(NH + h + 1) * 64] * (-lam * scale))
            wk_cols.append(Wk[:, h * 64:(h + 1) * 64])
            wk_cols.append(Wk[:, (NH + h) * 64:(NH + h + 1) * 64])
        wv_pad = np.zeros((H, NHC * 65), dtype=np.float32)
        for j, h in enumerate(heads):
            wv_pad[:, j * 65:j * 65 + 64] = Wv[:, h * 64:(h + 1) * 64]
        heads = list(heads)
        wo_sel = np.zeros((128, 2, S), dtype=np.float32)  # head-pair stacked rows
        for p in range(2):
            h0, h1 = heads[2 * p], heads[2 * p + 1]
            wo_sel[0:64, p] = Wo[h0 * 64:(h0 + 1) * 64, :]
            wo_sel[64:128, p] = Wo[h1 * 64:(h1 + 1) * 64, :]
        wqa = np.concatenate(wq_cols, axis=1)
        wka = np.concatenate(wk_cols, axis=1)
        in_maps.append({
            "hT": hTb[b],
            "wq": np.ascontiguousarray(
                wqa.reshape(16, 128, 4, 128).transpose(1, 2, 0, 3)).astype(BF),
            "wk": np.ascontiguousarray(
                wka.reshape(16, 128, 4, 128).transpose(1, 2, 0, 3)).astype(BF),
            "wv": np.ascontiguousarray(
                wv_pad.reshape(16, 128, 260).transpose(1, 0, 2)).astype(BF),
            "wo": np.ascontiguousarray(wo_sel).astype(BF),
        })
    return in_maps


def _mask_is_causal(attention_mask):
    m = np.asarray(attention_mask)
    if m.shape != (2, 1, S, S):
        return False
    neg = np.float32(np.finfo(np.float32).min)
    tri = np.tril(np.ones((S, S), dtype=bool))
    expect = np.where(tri, np.float32(0.0), neg)
    return all(np.array_equal(m[b, 0], expect) for b in range(m.shape[0]))


def _fallback(hidden_states, attention_mask, Wq, Wk, Wv, Wo, lambda_param):
    hs = hidden_states.astype(np.float32)
    lam = math.tanh(math.log1p(math.exp(float(lambda_param))))
    scaling = HD ** -0.5
    B = hs.shape[0]
    out = np.empty((B, S, H), dtype=np.float32)
    for b in range(B):
        q_all = (hs[b] @ Wq).reshape(S, 2 * NH, HD).transpose(1, 0, 2)
        k_all = (hs[b] @ Wk).reshape(S, 2 * NH, HD).transpose(1, 0, 2)
        v = (hs[b] @ Wv).reshape(S, NH, HD).transpose(1, 0, 2)
        acc = np.zeros((S, H), dtype=np.float32)
        for h in range(NH):
            s1 = q_all[h] @ k_all[h].T
            s2 = q_all[NH + h] @ k_all[NH + h].T
            sc = (s1 - lam * s2) * scaling + attention_mask[b, 0]
            sc -= sc.max(axis=-1, keepdims=True)
            p = np.exp(sc)
            p /= p.sum(axis=-1, keepdims=True)
            acc += (p @ v[h]) @ Wo[h * 64:(h + 1) * 64]
        out[b] = acc
    return out


def _run(inputs, trace=False):
    from concourse.bass_utils import run_bass_kernel_spmd

    hidden_states = np.asarray(inputs["hidden_states"], dtype=np.float32)
    attention_mask = np.asarray(inputs["attention_mask"], dtype=np.float32)
    Wq = np.asarray(inputs["Wq"], dtype=np.float32)
    Wk = np.asarray(inputs["Wk"], dtype=np.float32)
    Wv = np.asarray(inputs["Wv"], dtype=np.float32)
    Wo = np.asarray(inputs["Wo"], dtype=np.float32)
    lam_p = inputs["lambda_param"]

    if not _mask_is_causal(attention_mask):
        return _fallback(hidden_states, attention_mask, Wq, Wk, Wv, Wo, lam_p), None

    in_maps = _prep_in_maps(hidden_states, Wq, Wk, Wv, Wo, lam_p)
    nc = _get_nc()
    res = run_bass_kernel_spmd(nc, in_maps, list(range(8)), trace=trace)
    out = np.empty((2, S, H), dtype=np.float32)
    for b in range(2):
        acc = res.results[4 * b]["out"].astype(np.float32)
        for g in range(1, 4):
            acc = acc + res.results[4 * b + g]["out"].astype(np.float32)
        out[b] = acc
    return out, res


def kernel(**inputs):
    out, _ = _run(inputs, trace=False)
    return out


# revision 45
# speedup vs baseline: 1.0056x; 1.0049x over previous
"""Differential attention kernel for 8 Trainium2 NeuronCores.

Sharding: batch x head-group. Core c handles batch b = c//4, heads
[4g, 4g+4) with g = c%4. Each core computes Q/K/V projections for its
heads over the full sequence, causal differential attention, and its
partial O-projection; the host sums the 4 partials per batch.

Differential attention trick: score = (q1.k1 - lam*q2.k2) * scale is a
single K=128 matmul with stacked [q1*scale; -lam*scale*q2] and [k1; k2]
head vectors (scales folded into the projection weights on the host).

Pipeline: heads are processed in sequence; while head h runs its
exp-bound attention on the Scalar engine, the PE executes head h+1's
Q/K projection chunks as filler, so the ~92us of exp hides behind
matmuls instead of serializing after them. Head 3's filler is the
O-projection of already-finished query chunks.

Softmax: scores are computed transposed (keys on partitions, queries
free), exp'd without max subtraction (inputs are bounded), and the
denominator comes from a ones-column appended to V in the P@V matmul.
Normalization per (head, chunk): DVE approx-reciprocal on the PSUM
denominator row, GpSimd partition-broadcast, DVE multiply. Causality
is structural (upper blocks skipped, diagonal score columns trimmed,
in-block triangle zeroed post-exp on DVE), which the host validates
against the attention_mask input before dispatch.
"""
import math
from collections import deque
from contextlib import ExitStack

import numpy as np
import ml_dtypes

S = 2048
H = 2048
NH = 16
HD = 64
NHC = 4          # heads per core
BF = ml_dtypes.bfloat16

_CACHED_NC = None
_DEBUG_TAPS = False


def _build_nc():
    import concourse.mybir as mybir
    import concourse.tile as tile
    from concourse import bacc

    bf16 = mybir.dt.bfloat16
    f32 = mybir.dt.float32
    AF = mybir.ActivationFunctionType

    nc = bacc.Bacc(None, target_bir_lowering=False)
    # inputs pre-arranged on host into partition-major layouts so every
    # DMA below is a contiguous stream
    hT = nc.declare_dram_parameter("hT", [128, S // 512, H // 128, 512], bf16, isOutput=False)
    wq = nc.declare_dram_parameter("wq", [128, NHC, H // 128, 128], bf16, isOutput=False)
    wk = nc.declare_dram_parameter("wk", [128, NHC, H // 128, 128], bf16, isOutput=False)
    wv = nc.declare_dram_parameter("wv", [128, H // 128, NHC * 65], bf16, isOutput=False)
    wo = nc.declare_dram_parameter("wo", [128, 2, S], bf16, isOutput=False)
    out = nc.declare_dram_parameter("out", [S, H], bf16, isOutput=True)
    if _DEBUG_TAPS:
        dbg_qT = nc.declare_dram_parameter("dbg_qT", [128, NHC, S], bf16, isOutput=True)
        dbg_kT = nc.declare_dram_parameter("dbg_kT", [128, NHC, S], bf16, isOutput=True)
        dbg_v4 = nc.declare_dram_parameter("dbg_v4", [128, S // 128, NHC * 65], bf16, isOutput=True)
        dbg_avt = nc.declare_dram_parameter("dbg_avt", [128, 2, S], bf16, isOutput=True)
        dbg_rec = nc.declare_dram_parameter("dbg_rec", [NHC * 4, 512], mybir.dt.float32, isOutput=True)
        dbg_den = nc.declare_dram_parameter("dbg_den", [NHC * 4, 512], mybir.dt.float32, isOutput=True)

    KT = H // 128    # 16 contraction tiles for projections
    NQ = S // 512    # 4 query chunks
    NS = S // 128    # 16 seq tiles

    with tile.TileContext(nc) as tc:
        with ExitStack() as ctx:
            # ---- persistent SBUF ----
            sb = ctx.enter_context(tc.tile_pool(name="sb", bufs=1))
            ht_all = sb.tile([128, KT, S], bf16)          # hidden^T
            wq_sb = sb.tile([128, NHC, KT, 128], bf16)
            wk_sb = sb.tile([128, NHC, KT, 128], bf16)
            wv_sb = sb.tile([128, KT, NHC * 65], bf16)
            wo_sb = sb.tile([128, 2, S], bf16)            # head-pair stacked Wo rows
            qT = sb.tile([128, NHC, S], bf16)             # [q1*s; -lam*s*q2] per head
            kT = sb.tile([128, NHC, S], bf16)             # [k1; k2] per head
            v4 = sb.tile([128, NS, NHC * 65], bf16)       # V tiles + ones cols
            avt = sb.tile([128, 2, S], bf16)              # attn_out^T, head pairs stacked
            # warm the ACT exp table while DMAs stream in
            warm = sb.tile([1, 16], f32)
            nc.vector.memset(warm[:], 0.0)
            nc.scalar.activation(warm[:], warm[:], AF.Exp)
            # lower-triangle (key <= query) 0/1 mask for diagonal blocks
            tri = sb.tile([128, 128], bf16)
            nc.gpsimd.memset(tri[:], 1.0)
            nc.gpsimd.affine_select(
                out=tri[:], in_=tri[:],
                compare_op=mybir.AluOpType.is_ge, fill=0.0,
                base=0, channel_multiplier=-1, pattern=[[1, 128]])

            # ---- input DMAs: few fat descriptors (Sync descriptor issue is
            # ~0.6us each and was pacing the whole projection ramp), ordered
            # by first use: head-0 weights, hidden k-groups, then the rest ----
            # hidden streams by query-column chunk: all 16 k-tiles of 512
            # queries per descriptor, so head-0's chunk-c projection AND
            # attention can start as soon as column chunk c lands
            nc.sync.dma_start(out=wq_sb[:, 0, :, :], in_=wq[:, 0, :, :])
            nc.sync.dma_start(out=wk_sb[:, 0, :, :], in_=wk[:, 0, :, :])
            nc.sync.dma_start(out=ht_all[:, :, 0:512], in_=hT[:, 0, :, :])
            nc.sync.dma_start(out=wv_sb[:], in_=wv[:, :, :])
            for c in range(1, NQ):
                nc.sync.dma_start(out=ht_all[:, :, c * 512:(c + 1) * 512],
                                  in_=hT[:, c, :, :])
            nc.sync.dma_start(out=wq_sb[:, 1, :, :], in_=wq[:, 1, :, :])
            nc.sync.dma_start(out=wk_sb[:, 1, :, :], in_=wk[:, 1, :, :])

            pt_pool = ctx.enter_context(tc.tile_pool(name="ptw", bufs=4))
            nrm_work = ctx.enter_context(tc.tile_pool(name="nrmw", bufs=2))
            orow_pool = ctx.enter_context(tc.tile_pool(name="orow", bufs=4))

            atp = ctx.enter_context(tc.tile_pool(name="atp", bufs=2, space="PSUM"))
            wkp = ctx.enter_context(tc.tile_pool(name="wkp", bufs=2, space="PSUM"))

            # warm the PE clock (HAM) with junk matmuls while DMAs land;
            # first on `warm` (ready earliest), then on tri
            wj = wkp.tile([128, 512], f32, tag="work", name="warmp")
            for _ in range(24):
                nc.tensor.matmul(wj[0:16, 0:16], lhsT=warm[:], rhs=warm[:],
                                 start=True, stop=True)
            for _ in range(44):
                nc.tensor.matmul(wj[:, 0:128], lhsT=tri[:], rhs=tri[:],
                                 start=True, stop=True)

            def v_unit(st):
                def run():
                    vp = wkp.tile([128, 512], f32, tag="work")
                    for k in range(KT):
                        nc.tensor.matmul(vp[:, 0:NHC * 65],
                                         lhsT=ht_all[:, k, st * 128:(st + 1) * 128],
                                         rhs=wv_sb[:, k, :],
                                         start=(k == 0), stop=(k == KT - 1))
                    nc.vector.tensor_copy(v4[:, st, :], vp[:, 0:NHC * 65])
                    for j in range(NHC):
                        nc.gpsimd.memset(v4[:, st, j * 65 + 64:j * 65 + 65], 1.0)
                return run

            def proj_unit(h, w_sb, dst, nj, evac=0):
                # two half-contraction pieces so filler slots stay fine-grained
                box = {}

                def run_a():
                    pp = wkp.tile([128, 512], f32, tag="work",
                                  name=f"pp{h}_{nj}_{0 if dst is qT else 1}")
                    box["pp"] = pp
                    for k in range(KT // 2):
                        nc.tensor.matmul(pp[:], lhsT=w_sb[:, h, k, :],
                                         rhs=ht_all[:, k, nj * 512:(nj + 1) * 512],
                                         start=(k == 0), stop=False)

                def run_b():
                    pp = box["pp"]
                    for k in range(KT // 2, KT):
                        nc.tensor.matmul(pp[:], lhsT=w_sb[:, h, k, :],
                                         rhs=ht_all[:, k, nj * 512:(nj + 1) * 512],
                                         start=False, stop=(k == KT - 1))
                    if evac:
                        nc.scalar.copy(dst[:, h, nj * 512:(nj + 1) * 512], pp[:])
                    else:
                        nc.vector.tensor_copy(dst[:, h, nj * 512:(nj + 1) * 512], pp[:])
                return run_a, run_b

            def oproj_unit(qi, nch, evac):
                def run():
                    op = wkp.tile([128, 512], f32, tag="work")
                    for p in range(2):
                        nc.tensor.matmul(op[:],
                                         lhsT=avt[:, p, qi * 128:(qi + 1) * 128],
                                         rhs=wo_sb[:, p, nch * 512:(nch + 1) * 512],
                                         start=(p == 0), stop=(p == 1))
                    orow = orows[qi]
                    if evac == 0:
                        nc.vector.tensor_copy(orow[:, nch * 512:(nch + 1) * 512], op[:])
                    else:
                        nc.scalar.copy(orow[:, nch * 512:(nch + 1) * 512], op[:])
                    done = orow_filled[qi] = orow_filled[qi] + 1
                    if done == 2:
                        nc.sync.dma_start(out=out[qi * 128:(qi + 1) * 128, 0:1024],
                                          in_=orow[:, 0:1024])
                    elif done == NQ:
                        nc.sync.dma_start(out=out[qi * 128:(qi + 1) * 128, 1024:2048],
                                          in_=orow[:, 1024:2048])
                return run

            orows = {}
            orow_filled = {}
            filler = deque()

            def attn_chunk(h, nj, slot_cb):
                qs = slice(nj * 512, (nj + 1) * 512)
                nblk = 4 * nj + 4
                av = atp.tile([65, 512], f32, tag="av", name=f"av{h}_{nj}")
                pv_lag = deque()   # (spans, pt) awaiting P@V, depth 2
                for kg in range(nblk // 2):
                    sc = atp.tile([128, 1024], f32, tag="sc", name=f"sc{h}_{nj}_{kg}")
                    pt = pt_pool.tile([128, 1024], bf16, tag="pt")
                    spans = []
                    for u in range(2):
                        ki = 2 * kg + u
                        uu = ki - 4 * nj  # >=0 on diagonal blocks
                        g0 = u * 512 + max(uu, 0) * 128
                        g1 = (u + 1) * 512
                        nc.tensor.matmul(sc[:, g0:g1],
                                         lhsT=kT[:, h, ki * 128:(ki + 1) * 128],
                                         rhs=qT[:, h, nj * 512 + max(uu, 0) * 128:(nj + 1) * 512],
                                         start=True, stop=True)
                        spans.append((g0, g1, uu))
                    if spans[0][1] == spans[1][0] == 512 and spans[0][0] == 0:
                        nc.scalar.activation(pt[:, 0:1024], sc[:, 0:1024], AF.Exp)
                    else:
                        for g0, g1, _ in spans:
                            nc.scalar.activation(pt[:, g0:g1], sc[:, g0:g1], AF.Exp)
                    for g0, _, uu in spans:
                        if uu >= 0:
                            # zero the in-block upper triangle (key > query)
                            nc.gpsimd.affine_select(
                                out=pt[:, g0:g0 + 128],
                                in_=pt[:, g0:g0 + 128],
                                compare_op=mybir.AluOpType.is_ge,
                                fill=0.0,
                                base=0,
                                channel_multiplier=-1,
                                pattern=[[1, 128]],
                            )
                    if len(pv_lag) == 2:
                        lag_spans, lag_pt = pv_lag.popleft()
                        for g0, g1, uu, ki in lag_spans:
                            nc.tensor.matmul(av[:, max(uu, 0) * 128:512],
                                             lhsT=v4[:, ki, h * 65:(h + 1) * 65],
                                             rhs=lag_pt[:, g0:g1],
                                             start=(ki == 0), stop=(ki == nblk - 1))
                    slot_cb()
                    pv_lag.append(([(g0, g1, uu, 2 * kg + i)
                                    for i, (g0, g1, uu) in enumerate(spans)], pt))
                while pv_lag:
                    lag_spans, lag_pt = pv_lag.popleft()
                    for g0, g1, uu, ki in lag_spans:
                        nc.tensor.matmul(av[:, max(uu, 0) * 128:512],
                                         lhsT=v4[:, ki, h * 65:(h + 1) * 65],
                                         rhs=lag_pt[:, g0:g1],
                                         start=(ki == 0), stop=(ki == nblk - 1))
                # normalize: row 64 of av is the softmax denominator
                den = nrm_work.tile([1, 512], f32, tag="den")
                nc.scalar.copy(den[:], av[64:65, :])
                rec = nrm_work.tile([1, 512], f32, tag="rec")
                nc.vector.reciprocal_approx_fast(rec[:], den[:])
                if _DEBUG_TAPS:
                    nc.sync.dma_start(out=dbg_rec[h * 4 + nj:h * 4 + nj + 1, :], in_=rec[:])
                    dent = nrm_work.tile([1, 512], f32, tag="dent")
                    nc.vector.tensor_copy(dent[:], av[64:65, :])
                    nc.sync.dma_start(out=dbg_den[h * 4 + nj:h * 4 + nj + 1, :], in_=dent[:])
                bcs = nrm_work.tile([64, 512], f32, tag="bcs")
                nc.gpsimd.partition_broadcast(bcs[:], rec[:])
                pair, odd = divmod(h, 2)
                nc.vector.tensor_mul(avt[64 * odd:64 * odd + 64, pair, qs],
                                     av[0:64, :], bcs[:])
                slot_cb()

            # ---- head 0: column-streamed ramp. Chunk-c projection, V
            # blocks, and attention run as soon as ht column-chunk c lands;
            # next column's units fill this chunk's attention slots ----
            for fn in proj_unit(0, wq_sb, qT, 0, evac=1):
                fn()
            for fn in proj_unit(0, wk_sb, kT, 0):
                fn()
            for st in range(4):
                v_unit(st)()
            for c in range(NQ):
                if c + 1 < NQ:
                    filler.extend(proj_unit(0, wq_sb, qT, c + 1, evac=1))
                    filler.extend(proj_unit(0, wk_sb, kT, c + 1))
                    for st in range(4 * (c + 1), 4 * (c + 1) + 4):
                        filler.append(v_unit(st))
                else:
                    # head-2 weights stream during head-0 attention
                    nc.sync.dma_start(out=wq_sb[:, 2, :, :], in_=wq[:, 2, :, :])
                    nc.sync.dma_start(out=wk_sb[:, 2, :, :], in_=wk[:, 2, :, :])
                    for w_sb, dst in ((wq_sb, qT), (wk_sb, kT)):
                        for nj in range(NQ):
                            filler.extend(proj_unit(1, w_sb, dst, nj))
                cslots = [2 * c + 3]

                def c_cb(cslots=cslots):
                    n = 0 if not filler else -(-len(filler) // max(1, cslots[0]))
                    for _ in range(min(n, len(filler))):
                        filler.popleft()()
                    cslots[0] -= 1
                attn_chunk(0, c, c_cb)

            # ---- heads 1-3 ----
            TOT_SLOTS = sum(2 * nj + 2 + 1 for nj in range(NQ))
            for h in range(1, NHC):
                # deferred weight streams off the ramp-critical window
                if h == 1:
                    nc.sync.dma_start(out=wq_sb[:, 3, :, :], in_=wq[:, 3, :, :])
                    nc.sync.dma_start(out=wk_sb[:, 3, :, :], in_=wk[:, 3, :, :])
                    nc.sync.dma_start(out=wo_sb[:], in_=wo[:, :, :])
                # queue next head's Q/K projection as PE filler under this
                # head's exp-bound attention
                if h + 1 < NHC:
                    for w_sb, dst in ((wq_sb, qT), (wk_sb, kT)):
                        for nj in range(NQ):
                            filler.extend(proj_unit(h + 1, w_sb, dst, nj))
                n0 = len(filler)
                plan = [0] * TOT_SLOTS
                for i in range(n0):
                    plan[i * TOT_SLOTS // n0] += 1
                state = [0, TOT_SLOTS]

                def h_cb(h=h, plan=plan, state=state):
                    slot_no, slots = state
                    n = plan[slot_no]
                    if h == NHC - 1 and filler:
                        n += -(-len(filler) // max(1, slots))
                    for _ in range(min(n, len(filler))):
                        filler.popleft()()
                    state[0] += 1
                    state[1] -= 1
                for nj in range(NQ):
                    attn_chunk(h, nj, h_cb)
                    if h == NHC - 1:
                        # O-projection of this finished chunk becomes filler
                        for qi in range(4 * nj, 4 * nj + 4):
                            orows[qi] = orow_pool.tile([128, S], bf16, tag="orow",
                                                       name=f"or{qi}")
                            orow_filled[qi] = 0
                            for nch in range(NQ):
                                filler.append(oproj_unit(qi, nch, (qi + nch) % 3 == 0))
            while filler:
                filler.popleft()()
            if _DEBUG_TAPS:
                nc.sync.dma_start(out=dbg_qT[:], in_=qT[:])
                nc.sync.dma_start(out=dbg_kT[:], in_=kT[:])
                nc.sync.dma_start(out=dbg_v4[:], in_=v4[:])
                nc.sync.dma_start(out=dbg_avt[:], in_=avt[:])
    return nc


def _get_nc():
    global _CACHED_NC
    if _CACHED_NC is None:
        nc = _build_nc()
        if not nc.is_finalized():
            nc.finalize()
        _CACHED_NC = nc
    return _CACHED_NC


def _prep_in_maps(hidden_states, Wq, Wk, Wv, Wo, lambda_param):
    lam = math.tanh(math.log1p(math.exp(float(lambda_param))))
    scale = HD ** -0.5
    in_maps = []
    # [p, col-chunk, k, s]: DMA-contiguous per 512-query column chunk
    hTb = [np.ascontiguousarray(
        hidden_states[b].T.reshape(16, 128, 4, 512).transpose(1, 2, 0, 3)
    ).astype(BF) for b in range(2)]
    for core in range(8):
        b, g = divmod(core, 4)
        heads = range(NHC * g, NHC * g + NHC)
        wq_cols, wk_cols = [], []
        for h in heads:
            wq_cols.append(Wq[:, h * 64:(h + 1) * 64] * scale)
            wq_cols.append(Wq[:, (NH + h) * 64:(NH + h + 1) * 64] * (-lam * scale))
            wk_cols.append(Wk[:, h * 64:(h + 1) * 64])
            wk_cols.append(Wk[:, (NH + h) * 64:(NH + h + 1) * 64])
        wv_pad = np.zeros((H, NHC * 65), dtype=np.float32)
        for j, h in enumerate(heads):
            wv_pad[:, j * 65:j * 65 + 64] = Wv[:, h * 64:(h + 1) * 64]
        heads = list(heads)
        wo_sel = np.zeros((128, 2, S), dtype=np.float32)  # head-pair stacked rows
        for p in range(2):
            h0, h1 = heads[2 * p], heads[2 * p + 1]
            wo_sel[0:64, p] = Wo[h0 * 64:(h0 + 1) * 64, :]
            wo_sel[64:128, p] = Wo[h1 * 64:(h1 + 1) * 64, :]
        wqa = np.concatenate(wq_cols, axis=1)
        wka = np.concatenate(wk_cols, axis=1)
        in_maps.append({
            "hT": hTb[b],
            "wq": np.ascontiguousarray(
                wqa.reshape(16, 128, 4, 128).transpose(1, 2, 0, 3)).astype(BF),
            "wk": np.ascontiguousarray(
                wka.reshape(16, 128, 4, 128).transpose(1, 2, 0, 3)).astype(BF),
            "wv": np.ascontiguousarray(
                wv_pad.reshape(16, 128, 260).transpose(1, 0, 2)).astype(BF),
            "wo": np.ascontiguousarray(wo_sel).astype(BF),
        })
    return in_maps


def _mask_is_causal(attention_mask):
    m = np.asarray(attention_mask)
    if m.shape != (2, 1, S, S):
        return False
    neg = np.float32(np.finfo(np.float32).min)
    tri = np.tril(np.ones((S, S), dtype=bool))
    expect = np.where(tri, np.float32(0.0), neg)
    return all(np.array_equal(m[b, 0], expect) for b in range(m.shape[0]))


def _fallback(hidden_states, attention_mask, Wq, Wk, Wv, Wo, lambda_param):
    hs = hidden_states.astype(np.float32)
    lam = math.tanh(math.log1p(math.exp(float(lambda_param))))
    scaling = HD ** -0.5
    B = hs.shape[0]
    out = np.empty((B, S, H), dtype=np.float32)
    for b in range(B):
        q_all = (hs[b] @ Wq).reshape(S, 2 * NH, HD).transpose(1, 0, 2)
        k_all = (hs[b] @ Wk).reshape(S, 2 * NH, HD).transpose(1, 0, 2)
        v = (hs[b] @ Wv).reshape(S, NH, HD).transpose(1, 0, 2)
        acc = np.zeros((S, H), dtype=np.float32)
        for h in range(NH):
            s1 = q_all[h] @ k_all[h].T
            s2 = q_all[NH + h] @ k_all[NH + h].T
            sc = (s1 - lam * s2) * scaling + attention_mask[b, 0]
            sc -= sc.max(axis=-1, keepdims=True)
            p = np.exp(sc)
            p /= p.sum(axis=-1, keepdims=True)
            acc += (p @ v[h]) @ Wo[h * 64:(h + 1) * 64]
        out[b] = acc
    return out


def _run(inputs, trace=False):
    from concourse.bass_utils import run_bass_kernel_spmd

    hidden_states = np.asarray(inputs["hidden_states"], dtype=np.float32)
    attention_mask = np.asarray(inputs["attention_mask"], dtype=np.float32)
    Wq = np.asarray(inputs["Wq"], dtype=np.float32)
    Wk = np.asarray(inputs["Wk"], dtype=np.float32)
    Wv = np.asarray(inputs["Wv"], dtype=np.float32)
    Wo = np.asarray(inputs["Wo"], dtype=np.float32)
    lam_p = inputs["lambda_param"]

    if not _mask_is_causal(attention_mask):
        return _fallback(hidden_states, attention_mask, Wq, Wk, Wv, Wo, lam_p), None

    in_maps = _prep_in_maps(hidden_states, Wq, Wk, Wv, Wo, lam_p)
    nc = _get_nc()
    res = run_bass_kernel_spmd(nc, in_maps, list(range(8)), trace=trace)
    out = np.empty((2, S, H), dtype=np.float32)
    for b in range(2):
        acc = res.results[4 * b]["out"].astype(np.float32)
        for g in range(1, 4):
            acc = acc + res.results[4 * b + g]["out"].astype(np.float32)
        out[b] = acc
    return out, res


def kernel(**inputs):
    out, _ = _run(inputs, trace=False)
    return out
